# revision 44
# baseline (speedup 1.0000x reference)
"""ODE-RNN decoder kernel for Trainium2 (8 NeuronCores, data-parallel).

Math per scan step (t = 0..98), per trajectory:
    y_ode = y + (tanh(y @ Wo1 + bo1) @ Wo2 + bo2) * dt_t
    z     = sigmoid(tanh([y_ode;x] @ Wz1 + bz1) @ Wz2 + bz2)
    r     = sigmoid(tanh([y_ode;x] @ Wr1 + br1) @ Wr2 + br2)
    h     = tanh(tanh([r*y_ode;x] @ Wh1 + bh1) @ Wh2 + bh2)
    y     = (1-z)*h + z*y_ode

Layout: feature-major on-chip ([feature, batch]); batch 8192 sharded 8 ways
data-parallel (1024/core, weights replicated), CH=2 chunks of 512 columns,
anti-phased half a step apart.  The recurrence is latency-bound, so the
kernel minimizes the per-step dependency cycle rather than engine
throughput:

- State is SPLIT as y_t = h_t + p_t - s_t (h = gate output, p = z*y_ode,
  s = z*h); y is never materialized in the loop.  Only s needs the
  just-produced h, so the post-tanh tail is ONE elementwise multiply.
- Moving operands are packed so each layer-1 GEMM is few matmuls:
    hs [128, n] = [s; h]  (tail s-mul writes rows 0:64 at base 0 reading
                           base-64 inputs; the h tanh writes rows 64:128 —
                           ACT may write a different partition base)
    px [97, n]  = [p; x; 1]  (p-mul, x-DMA, ones row — all mid-step)
  with stationaries W_hs = [-W; W], W_px = [W; Wx; bias]: biases ride the
  ones row, the subtraction rides negated weights.  The ODE update is
  folded in algebraically via Ez = dt*[Wo2;bo2]@W1y, so the gate path
  needs only the ODE tanh, never y_ode itself; the z/r layer-1 tanh is ONE
  fused instruction over a 2-bank PSUM tile.
- The exact y_ode is formed off-cycle for the elementwise ops, at BOTH
  partition bases (tensor-op inputs must share a base): base 0 pairs with
  r (sigmoid rows 0:64) for r*y_ode, base 64 pairs with z (rows 64:128)
  for z*y_ode and z*h.
- z/r layer-2 stack on partitions 0:64 (r) / 64:128 (z) of one PSUM bank
  so ONE sigmoid covers both; f32r matmuls cannot write PSUM base 64, so
  those weights and their moving tanh tile are bf16.
- PSUM tags are per-chunk (no cross-chunk WARs); the ODE layer-2 output
  p2 shares the l1 tag so the z/r layer-2 bank never waits on the y_ode
  stt readers (this WAR was worth ~1.4us/step).

Critical cycle: a5(h) -> s-mul -> hs-matmul -> a1(tode) -> ez -> a2 -> l2
-> a3(sigmoid) -> v2(r*y_ode) -> ph -> a4 -> ph2 -> a5, interleaved with
the other chunk's DVE/ACT bursts.
"""

import os
import sys

sys.path.insert(0, "/opt/trn_rl_repo")

from contextlib import ExitStack

import numpy as np

import concourse.bass as bass
import concourse.tile as tile
from concourse import bacc, mybir
from concourse.bass_utils import run_bass_kernel_spmd

N_TRAJ, T, DD, DL, NU = 8192, 100, 32, 64, 100
NSTEP = T - 1
NCORES = 8
B = N_TRAJ // NCORES  # 1024 per core
CH = int(os.environ.get("KCH", "2"))    # chunks in flight per core
NCH = B // CH                           # columns per chunk
assert NCH % 8 == 0 and NCH >= 256      # f32r matmul free-dim rules

F32 = mybir.dt.float32
F32R = mybir.dt.float32r
BF16 = mybir.dt.bfloat16
TANH = mybir.ActivationFunctionType.Tanh
SIG = mybir.ActivationFunctionType.Sigmoid
ADD = mybir.AluOpType.add
MULT = mybir.AluOpType.mult


def _build():
    nc = bacc.Bacc("TRN2", target_bir_lowering=False, debug=False)

    def din(name, shape, dt=F32R):
        return nc.dram_tensor(name, list(shape), dt, kind="ExternalInput")

    KS = DL + DD + 1  # 97: [s; x; 1]

    xs = din("xs", [NSTEP, DD, B])         # host: data[:,1:,:] transposed
    prior = din("prior", [DL, B])
    wo1hs = din("wo1hs", [2 * DL, NU])     # [-Wo1; Wo1] (s; h)
    wo1px = din("wo1px", [KS, NU])         # [Wo1; 0; bo1]
    wo2b = din("wo2b", [NU + 1, DL])       # [Wo2; bo2]
    wz1hs = din("wz1hs", [2 * DL, NU])     # [-Wz1y; Wz1y]
    wz1px = din("wz1px", [KS, NU])         # [Wz1y; Wz1x; bz1]
    ezb0 = din("ezb0", [NU + 1, NU])       # dt0 * [Wo2;bo2] @ Wz1y
    ezb = din("ezb", [NU + 1, NU])         # dtr * [Wo2;bo2] @ Wz1y
    wr1hs = din("wr1hs", [2 * DL, NU])
    wr1px = din("wr1px", [KS, NU])
    erb0 = din("erb0", [NU + 1, NU])
    erb = din("erb", [NU + 1, NU])
    wh1f = din("wh1f", [KS, NU])           # [Wh1y; Wh1x; bh1]
    wz2b = din("wz2b", [NU + 1, DL], BF16)  # [Wz2; bz2]
    wr2b = din("wr2b", [NU + 1, DL], BF16)  # [Wr2; br2]
    wh2b = din("wh2b", [NU + 1, DL])        # [Wh2; bh2]
    dts = din("dts", [DL, NSTEP], F32)      # exact per-step dt (y_ode path)
    zeros = din("zeros", [DL, B])           # p_0 = s_0 = 0
    ones = din("ones", [1, B])              # f32r ones rows
    ones16 = din("ones16", [1, 2 * B], BF16)
    yout = nc.dram_tensor("yout", [DL, B], F32R, kind="ExternalOutput")

    mmul = nc.tensor.matmul

    with tile.TileContext(nc) as tc, ExitStack() as ctx:
        singles = ctx.enter_context(tc.tile_pool(name="singles", bufs=1))
        psum = ctx.enter_context(tc.tile_pool(name="psum", bufs=1, space="PSUM"))

        def load(dr, shape, dt=F32R):
            t_ = singles.tile(shape, dt, tag=dr.name, name="s_" + dr.name)
            nc.sync.dma_start(t_[:], dr.ap())
            return t_

        s_wo1hs = load(wo1hs, [2 * DL, NU])
        s_wo1px = load(wo1px, [KS, NU])
        s_wo2b = load(wo2b, [NU + 1, DL])
        s_wz1hs = load(wz1hs, [2 * DL, NU])
        s_wz1px = load(wz1px, [KS, NU])
        s_ezb0 = load(ezb0, [NU + 1, NU])
        s_ezb = load(ezb, [NU + 1, NU])
        s_wr1hs = load(wr1hs, [2 * DL, NU])
        s_wr1px = load(wr1px, [KS, NU])
        s_erb0 = load(erb0, [NU + 1, NU])
        s_erb = load(erb, [NU + 1, NU])
        s_wh1f = load(wh1f, [KS, NU])
        s_wz2b = load(wz2b, [NU + 1, DL], BF16)
        s_wr2b = load(wr2b, [NU + 1, DL], BF16)
        s_wh2b = load(wh2b, [NU + 1, DL])
        s_dts = load(dts, [DL, NSTEP], F32)

        # per-chunk persistent state tiles
        st = {}
        for c in range(CH):
            cs = slice(c * NCH, (c + 1) * NCH)
            # double-buffered state: step t reads hs[t%2]/px[t%2]; a5 / p-mul
            # / s-mul / x-DMA of step t write into buffer (t+1)%2.
            # hs = [h; s] (both land late, ONE matmul per site reads them);
            # px = [p; x; 1] (all available early/mid-step).
            hs, px = [], []
            for j in range(2):
                hsj = singles.tile([2 * DL, NCH], F32R, tag=f"hs{c}_{j}",
                                   name=f"hs{c}_{j}")
                pxj = singles.tile([KS, NCH], F32R, tag=f"px{c}_{j}",
                                   name=f"px{c}_{j}")
                nc.sync.dma_start(pxj[DL + DD:KS, :], ones.ap()[:, 0:NCH])
                hs.append(hsj)
                px.append(pxj)
            nc.sync.dma_start(hs[0][DL:2 * DL, :], prior.ap()[:, cs])
            nc.sync.dma_start(hs[0][0:DL, :], zeros.ap()[:, cs])
            nc.sync.dma_start(px[0][0:DL, :], zeros.ap()[:, cs])
            nc.sync.dma_start(px[0][DL:DL + DD, :], xs.ap()[0, :, cs])
            rx = singles.tile([KS, NCH], F32R, tag=f"rx{c}", name=f"rx{c}")
            nc.sync.dma_start(rx[DL + DD:KS, :], ones.ap()[:, 0:NCH])
            tode = singles.tile([NU + 1, NCH], F32R, tag=f"to{c}", name=f"to{c}")
            nc.sync.dma_start(tode[NU:NU + 1, :], ones.ap()[:, 0:NCH])
            tzr = singles.tile([NU + 1, 2 * NCH], BF16, tag=f"tzr{c}",
                               name=f"tzr{c}")
            nc.sync.dma_start(tzr[NU:NU + 1, :], ones16.ap()[:, 0:2 * NCH])
            th = singles.tile([NU + 1, NCH], F32R, tag=f"th{c}", name=f"th{c}")
            nc.sync.dma_start(th[NU:NU + 1, :], ones.ap()[:, 0:NCH])
            st[c] = dict(
                hs=hs, px=px, rx=rx, tode=tode, tzr=tzr, th=th,
                w=singles.tile([2 * DL, NCH], F32, tag=f"w{c}", name=f"w{c}"),
                w2=singles.tile([DL, NCH], F32, tag=f"w2{c}", name=f"w2{c}"),
                yode=singles.tile([DL, NCH], F32, tag=f"yo{c}", name=f"yo{c}"),
                yo64=singles.tile([2 * DL, NCH], F32, tag=f"y6{c}",
                                  name=f"y6{c}"),
                szr=singles.tile([2 * DL, NCH], F32, tag=f"szr{c}",
                                 name=f"szr{c}"),
            )

        # Chunk 1 runs half a step behind chunk 0 (anti-phase): each
        # chunk's a1/a2 ACT burst overlaps the other's a3/a4/a5 burst
        # instead of colliding with the same-stage op on the in-order ACT
        # queue.  PSUM tags are per-chunk so no cross-chunk pool WARs.
        pz1 = {}

        def half1(c, t):
            """DMA, head (w1/w2), ODE l1 + tanh, gate l1 + fused tanh,
            ODE l2 and the exact y_ode."""
            s = st[c]
            cs = slice(c * NCH, (c + 1) * NCH)
            ez = s_ezb0 if t == 0 else s_ezb
            er = s_erb0 if t == 0 else s_erb
            cur_hs, cur_px = s["hs"][t % 2], s["px"][t % 2]
            if t + 1 < NSTEP:
                nc.sync.dma_start(s["px"][(t + 1) % 2][DL:DL + DD, :],
                                  xs.ap()[t + 1, :, cs])
            nc.sync.dma_start(s["rx"][DL:DL + DD, :], xs.ap()[t, :, cs])
            # w1 = p - s at base 64, w2 = w1 + h at base 0 (head, Pool)
            nc.gpsimd.tensor_sub(s["w"][DL:2 * DL, :],
                                 cur_px[0:DL, :].bitcast(F32),
                                 cur_hs[0:DL, :].bitcast(F32))
            nc.vector.tensor_add(s["w2"][:], s["w"][DL:2 * DL, :],
                                 cur_hs[DL:2 * DL, :].bitcast(F32))
            p1 = psum.tile([NU, NCH], F32, tag=f"l1{c}", name="p1", bufs=2)
            mmul(p1[:], s_wo1px[:], cur_px[:], start=True, stop=False)
            mmul(p1[:], s_wo1hs[:], cur_hs[:], start=False, stop=True)
            nc.scalar.activation(s["tode"][0:NU, :], p1[:], TANH)
            pzr1 = psum.tile([NU, 2 * NCH], F32, tag=f"l1w{c}",
                             name="pzr1", bufs=1)
            pz1[c] = pzr1
            mmul(pzr1[:, 0:NCH], s_wz1px[:], cur_px[:],
                 start=True, stop=False)
            mmul(pzr1[:, NCH:2 * NCH], s_wr1px[:], cur_px[:],
                 start=True, stop=False)
            mmul(pzr1[:, 0:NCH], s_wz1hs[:], cur_hs[:],
                 start=False, stop=False)
            mmul(pzr1[:, NCH:2 * NCH], s_wr1hs[:], cur_hs[:],
                 start=False, stop=False)
            with tc.high_priority():
                mmul(pzr1[:, 0:NCH], ez[:], s["tode"][:],
                     start=False, stop=True)
                mmul(pzr1[:, NCH:2 * NCH], er[:], s["tode"][:],
                     start=False, stop=True)
            p2 = psum.tile([2 * DL, NCH], F32, tag=f"l1{c}", name="p2",
                           bufs=2)
            mmul(p2[0:DL, :], s_wo2b[:], s["tode"][:])
            nc.scalar.activation(s["tzr"][0:NU, :], pzr1[:], TANH)
            # exact y_ode = dt*p2 + (h+p-s), at base 0 (for r) and base 64
            # (for z) — tensor-op inputs must share a partition base
            nc.vector.scalar_tensor_tensor(
                s["yode"][:], p2[0:DL, :], s_dts[:, t:t + 1],
                s["w2"][:], op0=MULT, op1=ADD)
            nc.vector.scalar_tensor_tensor(
                s["yo64"][DL:2 * DL, :], p2[0:DL, :], s_dts[:, t:t + 1],
                s["w2"][:], op0=MULT, op1=ADD)

        def half2(c, t):
            """zr layer 2 + sigmoid, h gate, s'/p' tail."""
            s = st[c]
            nxt_hs = s["hs"][(t + 1) % 2]
            nxt_px = s["px"][(t + 1) % 2]
            pzr2 = psum.tile([2 * DL, NCH], F32, tag=f"l1w{c}", name="pzr2",
                             bufs=1)
            with tc.high_priority():
                mmul(pzr2[0:DL, :], s_wr2b[:], s["tzr"][:, NCH:2 * NCH])
                mmul(pzr2[DL:2 * DL, :], s_wz2b[:], s["tzr"][:, 0:NCH])
            # rows 0:64 = r, 64:128 = z
            nc.scalar.activation(s["szr"][:], pzr2[:], SIG)
            # r*y_ode (all base 0)
            nc.vector.tensor_mul(s["rx"][0:DL, :], s["szr"][0:DL, :],
                                 s["yode"][:])
            ph = psum.tile([NU, NCH], F32, tag=f"l1{c}", name="ph", bufs=2)
            with tc.high_priority():
                mmul(ph[:], s_wh1f[:], s["rx"][:])
            nc.scalar.activation(s["th"][0:NU, :], ph[:], TANH)
            ph2 = psum.tile([2 * DL, NCH], F32, tag=f"l1{c}", name="ph2",
                            bufs=2)
            with tc.high_priority():
                mmul(ph2[0:DL, :], s_wh2b[:], s["th"][:])
            nc.scalar.activation(nxt_hs[DL:2 * DL, :], ph2[0:DL, :], TANH)
            # tail: s' = z*h_new — both inputs at base 64, out at base 0
            nc.vector.tensor_mul(nxt_hs[0:DL, :], s["szr"][DL:2 * DL, :],
                                 nxt_hs[DL:2 * DL, :].bitcast(F32))
            # p' = z*y_ode into px' rows 0:64 — emitted after s' so the
            # on-cycle tail multiply wins scheduler ties on DVE
            nc.vector.tensor_mul(nxt_px[0:DL, :], s["szr"][DL:2 * DL, :],
                                 s["yo64"][DL:2 * DL, :])

        assert CH == 2
        for t in range(NSTEP):
            half1(0, t)
            if t > 0:
                half2(1, t - 1)
            half2(0, t)
            half1(1, t)
        half2(1, NSTEP - 1)

        # y_final = h_99 + p_99 - s_99
        for c in range(CH):
            cs = slice(c * NCH, (c + 1) * NCH)
            s = st[c]
            jf = NSTEP % 2
            nc.gpsimd.tensor_sub(s["w"][DL:2 * DL, :],
                                 s["px"][jf][0:DL, :].bitcast(F32),
                                 s["hs"][jf][0:DL, :].bitcast(F32))
            yfin = singles.tile([DL, NCH], F32R, tag=f"yf{c}", name=f"yf{c}")
            nc.vector.tensor_add(yfin[:], s["w"][DL:2 * DL, :],
                                 s["hs"][jf][DL:2 * DL, :].bitcast(F32))
            nc.sync.dma_start(yout.ap()[:, cs], yfin[:])

    nc.compile()
    return nc


_NC_CACHE = None


def _get_nc():
    global _NC_CACHE
    if _NC_CACHE is None:
        _NC_CACHE = _build()
    return _NC_CACHE


def _prep_core_inputs(data, time_steps, prior, weights):
    """Host-side glue: shard + transpose into the kernel's layouts."""
    import ml_dtypes
    dts = np.concatenate([time_steps[1:2] - time_steps[0:1],
                          time_steps[:-2] - time_steps[1:-1]]).astype(np.float32)
    dts_b = np.ascontiguousarray(
        np.broadcast_to(dts[None, :], (DL, NSTEP))).astype(np.float32)
    (Wo1, bo1, Wo2, bo2, Wz1, bz1, Wz2, bz2,
     Wr1, br1, Wr2, br2, Wh1, bh1, Wh2, bh2) = weights

    def wb(W, b):
        return np.concatenate([W, b[None, :]], axis=0)

    def hsw(Wy):
        return np.concatenate([-Wy, Wy], axis=0)          # [128, 100] (s; h)

    def pxw(Wy, Wx, b):
        return np.concatenate([Wy, Wx, b[None, :]], axis=0)   # [97, 100]

    wo2b = wb(Wo2, bo2)                       # [101, 64]
    dt0 = float(dts[0])
    dtr = float(dts[1]) if NSTEP > 1 else dt0
    z32 = np.zeros((DD, NU), np.float32)
    shared = {
        "wo1hs": hsw(Wo1), "wo1px": pxw(Wo1, z32, bo1),
        "wo2b": wo2b,
        "wz1hs": hsw(Wz1[:DL]), "wz1px": pxw(Wz1[:DL], Wz1[DL:], bz1),
        "ezb0": dt0 * (wo2b @ Wz1[:DL]), "ezb": dtr * (wo2b @ Wz1[:DL]),
        "wr1hs": hsw(Wr1[:DL]), "wr1px": pxw(Wr1[:DL], Wr1[DL:], br1),
        "erb0": dt0 * (wo2b @ Wr1[:DL]), "erb": dtr * (wo2b @ Wr1[:DL]),
        "wh1f": wb(Wh1, bh1),
        "wh2b": wb(Wh2, bh2),
        "dts": dts_b,
        "zeros": np.zeros((DL, B), np.float32),
        "ones": np.ones((1, B), np.float32),
    }
    shared = {k: np.ascontiguousarray(v, dtype=np.float32)
              for k, v in shared.items()}
    shared["wz2b"] = wb(Wz2, bz2).astype(ml_dtypes.bfloat16)
    shared["wr2b"] = wb(Wr2, br2).astype(ml_dtypes.bfloat16)
    shared["ones16"] = np.ones((1, 2 * B), ml_dtypes.bfloat16)
    in_maps = []
    for i in range(NCORES):
        ts_ = slice(i * B, (i + 1) * B)
        xt = np.ascontiguousarray(
            data[ts_, 1:, :].transpose(1, 2, 0)).astype(np.float32)
        pr = np.ascontiguousarray(prior[ts_].T).astype(np.float32)
        in_maps.append({"xs": xt, "prior": pr, **shared})
    return in_maps


def kernel(data, time_steps, prior,
           Wo1, bo1, Wo2, bo2,
           Wz1, bz1, Wz2, bz2,
           Wr1, br1, Wr2, br2,
           Wh1, bh1, Wh2, bh2):
    data = np.asarray(data, dtype=np.float32)
    time_steps = np.asarray(time_steps, dtype=np.float32)
    prior = np.asarray(prior, dtype=np.float32)
    weights = [np.asarray(w, dtype=np.float32) for w in
               (Wo1, bo1, Wo2, bo2, Wz1, bz1, Wz2, bz2,
                Wr1, br1, Wr2, br2, Wh1, bh1, Wh2, bh2)]
    nc = _get_nc()
    in_maps = _prep_core_inputs(data, time_steps, prior, weights)
    res = run_bass_kernel_spmd(nc, in_maps, core_ids=list(range(NCORES)))
    out = np.empty((N_TRAJ, DL), dtype=np.float32)
    for i in range(NCORES):
        out[i * B:(i + 1) * B] = res.results[i]["yout"].T
    return out


# revision 45
# speedup vs baseline: 1.0660x; 1.0660x over previous
"""ODE-RNN decoder kernel for Trainium2 (8 NeuronCores, data-parallel).

Math per scan step (t = 0..98), per trajectory:
    y_ode = y + (tanh(y @ Wo1 + bo1) @ Wo2 + bo2) * dt_t
    z     = sigmoid(tanh([y_ode;x] @ Wz1 + bz1) @ Wz2 + bz2)
    r     = sigmoid(tanh([y_ode;x] @ Wr1 + br1) @ Wr2 + br2)
    h     = tanh(tanh([r*y_ode;x] @ Wh1 + bh1) @ Wh2 + bh2)
    y     = (1-z)*h + z*y_ode

Layout: feature-major on-chip ([feature, batch]); batch 8192 sharded 8 ways
data-parallel (1024/core, weights replicated), CH=2 chunks of 512 columns,
anti-phased half a step apart.  The recurrence is latency-bound, so the
kernel minimizes the per-step dependency cycle rather than engine
throughput:

- State is SPLIT as y_t = h_t + p_t - s_t (h = gate output, p = z*y_ode,
  s = z*h); y is never materialized in the loop.  Only s needs the
  just-produced h, so the post-tanh tail is ONE elementwise multiply.
- Moving operands are packed so each layer-1 GEMM is few matmuls:
    hs [128, n] = [s; h]  (tail s-mul writes rows 0:64 at base 0 reading
                           base-64 inputs; the h tanh writes rows 64:128 —
                           ACT may write a different partition base)
    px [97, n]  = [p; x; 1]  (p-mul, x-DMA, ones row — all mid-step)
  with stationaries W_hs = [-W; W], W_px = [W; Wx; bias]: biases ride the
  ones row, the subtraction rides negated weights.  The ODE update is
  folded in algebraically via Ez = dt*[Wo2;bo2]@W1y, so the gate path
  needs only the ODE tanh, never y_ode itself; the z/r layer-1 tanh is ONE
  fused instruction over a 2-bank PSUM tile.
- The exact y_ode is formed off-cycle for the elementwise ops, at BOTH
  partition bases (tensor-op inputs must share a base): base 0 pairs with
  r (sigmoid rows 0:64) for r*y_ode, base 64 pairs with z (rows 64:128)
  for z*y_ode and z*h.
- z/r layer-2 stack on partitions 0:64 (r) / 64:128 (z) of one PSUM bank
  so ONE sigmoid covers both; f32r matmuls cannot write PSUM base 64, so
  those weights and their moving tanh tile are bf16.
- PSUM tags are per-chunk (no cross-chunk WARs); the ODE layer-2 output
  p2 shares the l1 tag so the z/r layer-2 bank never waits on the y_ode
  stt readers (this WAR was worth ~1.4us/step).

Critical cycle: a5(h) -> s-mul -> hs-matmul -> a1(tode) -> ez -> a2 -> l2
-> a3(sigmoid) -> v2(r*y_ode) -> ph -> a4 -> ph2 -> a5, interleaved with
the other chunk's DVE/ACT bursts.
"""

import os
import sys

sys.path.insert(0, "/opt/trn_rl_repo")

from contextlib import ExitStack

import numpy as np

import concourse.bass as bass
import concourse.tile as tile
from concourse import bacc, mybir
from concourse.bass_utils import run_bass_kernel_spmd

N_TRAJ, T, DD, DL, NU = 8192, 100, 32, 64, 100
NSTEP = T - 1
NCORES = 8
B = N_TRAJ // NCORES  # 1024 per core
CH = int(os.environ.get("KCH", "2"))    # chunks in flight per core
NCH = B // CH                           # columns per chunk
assert NCH % 8 == 0 and NCH >= 256      # f32r matmul free-dim rules

F32 = mybir.dt.float32
F32R = mybir.dt.float32r
BF16 = mybir.dt.bfloat16
TANH = mybir.ActivationFunctionType.Tanh
SIG = mybir.ActivationFunctionType.Sigmoid
ADD = mybir.AluOpType.add
MULT = mybir.AluOpType.mult


def _build():
    nc = bacc.Bacc("TRN2", target_bir_lowering=False, debug=False)

    def din(name, shape, dt=F32R):
        return nc.dram_tensor(name, list(shape), dt, kind="ExternalInput")

    KS = DL + DD + 1  # 97: [s; x; 1]

    xs = din("xs", [NSTEP, DD, B])         # host: data[:,1:,:] transposed
    prior = din("prior", [DL, B])
    wo1hs = din("wo1hs", [2 * DL, NU])     # [-Wo1; Wo1] (s; h)
    wo1px = din("wo1px", [KS, NU])         # [Wo1; 0; bo1]
    wo2b = din("wo2b", [NU + 1, DL])       # [Wo2; bo2]
    wz1hs = din("wz1hs", [2 * DL, NU])     # [-Wz1y; Wz1y]
    wz1px = din("wz1px", [KS, NU])         # [Wz1y; Wz1x; bz1]
    ezb0 = din("ezb0", [NU + 1, NU])       # dt0 * [Wo2;bo2] @ Wz1y
    ezb = din("ezb", [NU + 1, NU])         # dtr * [Wo2;bo2] @ Wz1y
    wr1hs = din("wr1hs", [2 * DL, NU])
    wr1px = din("wr1px", [KS, NU])
    erb0 = din("erb0", [NU + 1, NU])
    erb = din("erb", [NU + 1, NU])
    wh1f = din("wh1f", [KS, NU])           # [Wh1y; Wh1x; bh1]
    wz2b = din("wz2b", [NU + 1, DL], BF16)  # [Wz2; bz2]
    wr2b = din("wr2b", [NU + 1, DL], BF16)  # [Wr2; br2]
    wh2b = din("wh2b", [NU + 1, DL])        # [Wh2; bh2]
    dts = din("dts", [DL, NSTEP], F32)      # exact per-step dt (y_ode path)
    zeros = din("zeros", [DL, B])           # p_0 = s_0 = 0
    ones = din("ones", [1, B])              # f32r ones rows
    ones16 = din("ones16", [1, 2 * B], BF16)
    yout = nc.dram_tensor("yout", [DL, B], F32R, kind="ExternalOutput")

    mmul = nc.tensor.matmul

    with tile.TileContext(nc) as tc, ExitStack() as ctx:
        singles = ctx.enter_context(tc.tile_pool(name="singles", bufs=1))
        psum = ctx.enter_context(tc.tile_pool(name="psum", bufs=1, space="PSUM"))

        def load(dr, shape, dt=F32R):
            t_ = singles.tile(shape, dt, tag=dr.name, name="s_" + dr.name)
            nc.sync.dma_start(t_[:], dr.ap())
            return t_

        s_wo1hs = load(wo1hs, [2 * DL, NU])
        s_wo1px = load(wo1px, [KS, NU])
        s_wo2b = load(wo2b, [NU + 1, DL])
        s_wz1hs = load(wz1hs, [2 * DL, NU])
        s_wz1px = load(wz1px, [KS, NU])
        s_ezb0 = load(ezb0, [NU + 1, NU])
        s_ezb = load(ezb, [NU + 1, NU])
        s_wr1hs = load(wr1hs, [2 * DL, NU])
        s_wr1px = load(wr1px, [KS, NU])
        s_erb0 = load(erb0, [NU + 1, NU])
        s_erb = load(erb, [NU + 1, NU])
        s_wh1f = load(wh1f, [KS, NU])
        s_wz2b = load(wz2b, [NU + 1, DL], BF16)
        s_wr2b = load(wr2b, [NU + 1, DL], BF16)
        s_wh2b = load(wh2b, [NU + 1, DL])
        s_dts = load(dts, [DL, NSTEP], F32)

        # per-chunk persistent state tiles
        st = {}
        for c in range(CH):
            cs = slice(c * NCH, (c + 1) * NCH)
            # double-buffered state: step t reads hs[t%2]/px[t%2]; a5 / p-mul
            # / s-mul / x-DMA of step t write into buffer (t+1)%2.
            # hs = [h; s] (both land late, ONE matmul per site reads them);
            # px = [p; x; 1] (all available early/mid-step).
            hs, px = [], []
            for j in range(2):
                hsj = singles.tile([2 * DL, NCH], F32R, tag=f"hs{c}_{j}",
                                   name=f"hs{c}_{j}")
                pxj = singles.tile([KS, NCH], F32R, tag=f"px{c}_{j}",
                                   name=f"px{c}_{j}")
                nc.sync.dma_start(pxj[DL + DD:KS, :], ones.ap()[:, 0:NCH])
                hs.append(hsj)
                px.append(pxj)
            nc.sync.dma_start(hs[0][DL:2 * DL, :], prior.ap()[:, cs])
            nc.sync.dma_start(hs[0][0:DL, :], zeros.ap()[:, cs])
            nc.sync.dma_start(px[0][0:DL, :], zeros.ap()[:, cs])
            nc.sync.dma_start(px[0][DL:DL + DD, :], xs.ap()[0, :, cs])
            rx = singles.tile([KS, NCH], F32R, tag=f"rx{c}", name=f"rx{c}")
            nc.sync.dma_start(rx[DL + DD:KS, :], ones.ap()[:, 0:NCH])
            tode = singles.tile([NU + 1, NCH], F32R, tag=f"to{c}", name=f"to{c}")
            nc.sync.dma_start(tode[NU:NU + 1, :], ones.ap()[:, 0:NCH])
            tzr = singles.tile([NU + 1, 2 * NCH], BF16, tag=f"tzr{c}",
                               name=f"tzr{c}")
            nc.sync.dma_start(tzr[NU:NU + 1, :], ones16.ap()[:, 0:2 * NCH])
            th = singles.tile([NU + 1, NCH], F32R, tag=f"th{c}", name=f"th{c}")
            nc.sync.dma_start(th[NU:NU + 1, :], ones.ap()[:, 0:NCH])
            st[c] = dict(
                hs=hs, px=px, rx=rx, tode=tode, tzr=tzr, th=th,
                w=singles.tile([2 * DL, NCH], F32, tag=f"w{c}", name=f"w{c}"),
                w2=singles.tile([DL, NCH], F32, tag=f"w2{c}", name=f"w2{c}"),
                yode=singles.tile([DL, NCH], F32, tag=f"yo{c}", name=f"yo{c}"),
                yo64=singles.tile([2 * DL, NCH], F32, tag=f"y6{c}",
                                  name=f"y6{c}"),
                szr=singles.tile([2 * DL, NCH], F32, tag=f"szr{c}",
                                 name=f"szr{c}"),
            )

        # Chunk 1 runs half a step behind chunk 0 (anti-phase): each
        # chunk's a1/a2 ACT burst overlaps the other's a3/a4/a5 burst
        # instead of colliding with the same-stage op on the in-order ACT
        # queue.  PSUM tags are per-chunk so no cross-chunk pool WARs.
        pz1 = {}

        def half1(c, t):
            """DMA, head (w1/w2), ODE l1 + tanh, gate l1 + fused tanh,
            ODE l2 and the exact y_ode."""
            s = st[c]
            cs = slice(c * NCH, (c + 1) * NCH)
            ez = s_ezb0 if t == 0 else s_ezb
            er = s_erb0 if t == 0 else s_erb
            cur_hs, cur_px = s["hs"][t % 2], s["px"][t % 2]
            if t + 1 < NSTEP:
                nc.sync.dma_start(s["px"][(t + 1) % 2][DL:DL + DD, :],
                                  xs.ap()[t + 1, :, cs])
            nc.sync.dma_start(s["rx"][DL:DL + DD, :], xs.ap()[t, :, cs])
            # w1 = p - s at base 64, w2 = w1 + h at base 0 (head, Pool)
            nc.gpsimd.tensor_sub(s["w"][DL:2 * DL, :],
                                 cur_px[0:DL, :].bitcast(F32),
                                 cur_hs[0:DL, :].bitcast(F32))
            nc.gpsimd.tensor_add(s["w2"][:], s["w"][DL:2 * DL, :],
                                 cur_hs[DL:2 * DL, :].bitcast(F32))
            p1 = psum.tile([NU, NCH], F32, tag=f"l1{c}", name="p1", bufs=2)
            mmul(p1[:], s_wo1px[:], cur_px[:], start=True, stop=False)
            mmul(p1[:], s_wo1hs[:], cur_hs[:], start=False, stop=True)
            nc.scalar.activation(s["tode"][0:NU, :], p1[:], TANH)
            pzr1 = psum.tile([NU, 2 * NCH], F32, tag=f"l1w{c}",
                             name="pzr1", bufs=1)
            pz1[c] = pzr1
            mmul(pzr1[:, 0:NCH], s_wz1px[:], cur_px[:],
                 start=True, stop=False)
            mmul(pzr1[:, NCH:2 * NCH], s_wr1px[:], cur_px[:],
                 start=True, stop=False)
            mmul(pzr1[:, 0:NCH], s_wz1hs[:], cur_hs[:],
                 start=False, stop=False)
            mmul(pzr1[:, NCH:2 * NCH], s_wr1hs[:], cur_hs[:],
                 start=False, stop=False)
            with tc.high_priority():
                mmul(pzr1[:, 0:NCH], ez[:], s["tode"][:],
                     start=False, stop=True)
                mmul(pzr1[:, NCH:2 * NCH], er[:], s["tode"][:],
                     start=False, stop=True)
            p2 = psum.tile([2 * DL, NCH], F32, tag=f"l1{c}", name="p2",
                           bufs=2)
            mmul(p2[0:DL, :], s_wo2b[:], s["tode"][:])
            nc.scalar.activation(s["tzr"][0:NU, :], pzr1[:], TANH)
            # exact y_ode = dt*p2 + (h+p-s), at base 0 (for r) and base 64
            # (for z) — tensor-op inputs must share a partition base
            nc.vector.scalar_tensor_tensor(
                s["yode"][:], p2[0:DL, :], s_dts[:, t:t + 1],
                s["w2"][:], op0=MULT, op1=ADD)
            nc.vector.scalar_tensor_tensor(
                s["yo64"][DL:2 * DL, :], p2[0:DL, :], s_dts[:, t:t + 1],
                s["w2"][:], op0=MULT, op1=ADD)

        def half2(c, t):
            """zr layer 2 + sigmoid, h gate, s'/p' tail."""
            s = st[c]
            nxt_hs = s["hs"][(t + 1) % 2]
            nxt_px = s["px"][(t + 1) % 2]
            pzr2 = psum.tile([2 * DL, NCH], F32, tag=f"l1w{c}", name="pzr2",
                             bufs=1)
            with tc.high_priority():
                mmul(pzr2[0:DL, :], s_wr2b[:], s["tzr"][:, NCH:2 * NCH])
                mmul(pzr2[DL:2 * DL, :], s_wz2b[:], s["tzr"][:, 0:NCH])
            # rows 0:64 = r, 64:128 = z
            nc.scalar.activation(s["szr"][:], pzr2[:], SIG)
            # r*y_ode (all base 0)
            nc.vector.tensor_mul(s["rx"][0:DL, :], s["szr"][0:DL, :],
                                 s["yode"][:])
            ph = psum.tile([NU, NCH], F32, tag=f"l1{c}", name="ph", bufs=2)
            with tc.high_priority():
                mmul(ph[:], s_wh1f[:], s["rx"][:])
            nc.scalar.activation(s["th"][0:NU, :], ph[:], TANH)
            ph2 = psum.tile([2 * DL, NCH], F32, tag=f"l1{c}", name="ph2",
                            bufs=2)
            with tc.high_priority():
                mmul(ph2[0:DL, :], s_wh2b[:], s["th"][:])
            nc.scalar.activation(nxt_hs[DL:2 * DL, :], ph2[0:DL, :], TANH)
            # tail: s' = z*h_new — both inputs at base 64, out at base 0
            nc.vector.tensor_mul(nxt_hs[0:DL, :], s["szr"][DL:2 * DL, :],
                                 nxt_hs[DL:2 * DL, :].bitcast(F32))
            # p' = z*y_ode into px' rows 0:64 — emitted after s' so the
            # on-cycle tail multiply wins scheduler ties on DVE
            nc.vector.tensor_mul(nxt_px[0:DL, :], s["szr"][DL:2 * DL, :],
                                 s["yo64"][DL:2 * DL, :])

        assert CH == 2
        for t in range(NSTEP):
            half1(0, t)
            if t > 0:
                half2(1, t - 1)
            half2(0, t)
            half1(1, t)
        half2(1, NSTEP - 1)

        # y_final = h_99 + p_99 - s_99
        for c in range(CH):
            cs = slice(c * NCH, (c + 1) * NCH)
            s = st[c]
            jf = NSTEP % 2
            nc.gpsimd.tensor_sub(s["w"][DL:2 * DL, :],
                                 s["px"][jf][0:DL, :].bitcast(F32),
                                 s["hs"][jf][0:DL, :].bitcast(F32))
            yfin = singles.tile([DL, NCH], F32R, tag=f"yf{c}", name=f"yf{c}")
            nc.vector.tensor_add(yfin[:], s["w"][DL:2 * DL, :],
                                 s["hs"][jf][DL:2 * DL, :].bitcast(F32))
            nc.sync.dma_start(yout.ap()[:, cs], yfin[:])

    nc.compile()
    return nc


_NC_CACHE = None


def _get_nc():
    global _NC_CACHE
    if _NC_CACHE is None:
        _NC_CACHE = _build()
    return _NC_CACHE


def _prep_core_inputs(data, time_steps, prior, weights):
    """Host-side glue: shard + transpose into the kernel's layouts."""
    import ml_dtypes
    dts = np.concatenate([time_steps[1:2] - time_steps[0:1],
                          time_steps[:-2] - time_steps[1:-1]]).astype(np.float32)
    dts_b = np.ascontiguousarray(
        np.broadcast_to(dts[None, :], (DL, NSTEP))).astype(np.float32)
    (Wo1, bo1, Wo2, bo2, Wz1, bz1, Wz2, bz2,
     Wr1, br1, Wr2, br2, Wh1, bh1, Wh2, bh2) = weights

    def wb(W, b):
        return np.concatenate([W, b[None, :]], axis=0)

    def hsw(Wy):
        return np.concatenate([-Wy, Wy], axis=0)          # [128, 100] (s; h)

    def pxw(Wy, Wx, b):
        return np.concatenate([Wy, Wx, b[None, :]], axis=0)   # [97, 100]

    wo2b = wb(Wo2, bo2)                       # [101, 64]
    dt0 = float(dts[0])
    dtr = float(dts[1]) if NSTEP > 1 else dt0
    z32 = np.zeros((DD, NU), np.float32)
    shared = {
        "wo1hs": hsw(Wo1), "wo1px": pxw(Wo1, z32, bo1),
        "wo2b": wo2b,
        "wz1hs": hsw(Wz1[:DL]), "wz1px": pxw(Wz1[:DL], Wz1[DL:], bz1),
        "ezb0": dt0 * (wo2b @ Wz1[:DL]), "ezb": dtr * (wo2b @ Wz1[:DL]),
        "wr1hs": hsw(Wr1[:DL]), "wr1px": pxw(Wr1[:DL], Wr1[DL:], br1),
        "erb0": dt0 * (wo2b @ Wr1[:DL]), "erb": dtr * (wo2b @ Wr1[:DL]),
        "wh1f": wb(Wh1, bh1),
        "wh2b": wb(Wh2, bh2),
        "dts": dts_b,
        "zeros": np.zeros((DL, B), np.float32),
        "ones": np.ones((1, B), np.float32),
    }
    shared = {k: np.ascontiguousarray(v, dtype=np.float32)
              for k, v in shared.items()}
    shared["wz2b"] = wb(Wz2, bz2).astype(ml_dtypes.bfloat16)
    shared["wr2b"] = wb(Wr2, br2).astype(ml_dtypes.bfloat16)
    shared["ones16"] = np.ones((1, 2 * B), ml_dtypes.bfloat16)
    in_maps = []
    for i in range(NCORES):
        ts_ = slice(i * B, (i + 1) * B)
        xt = np.ascontiguousarray(
            data[ts_, 1:, :].transpose(1, 2, 0)).astype(np.float32)
        pr = np.ascontiguousarray(prior[ts_].T).astype(np.float32)
        in_maps.append({"xs": xt, "prior": pr, **shared})
    return in_maps


def kernel(data, time_steps, prior,
           Wo1, bo1, Wo2, bo2,
           Wz1, bz1, Wz2, bz2,
           Wr1, br1, Wr2, br2,
           Wh1, bh1, Wh2, bh2):
    data = np.asarray(data, dtype=np.float32)
    time_steps = np.asarray(time_steps, dtype=np.float32)
    prior = np.asarray(prior, dtype=np.float32)
    weights = [np.asarray(w, dtype=np.float32) for w in
               (Wo1, bo1, Wo2, bo2, Wz1, bz1, Wz2, bz2,
                Wr1, br1, Wr2, br2, Wh1, bh1, Wh2, bh2)]
    nc = _get_nc()
    in_maps = _prep_core_inputs(data, time_steps, prior, weights)
    res = run_bass_kernel_spmd(nc, in_maps, core_ids=list(range(NCORES)))
    out = np.empty((N_TRAJ, DL), dtype=np.float32)
    for i in range(NCORES):
        out[i * B:(i + 1) * B] = res.results[i]["yout"].T
    return out
